# revision 11
# baseline (speedup 1.0000x reference)
"""Trainium2 Bass kernel for nn_DiagnosticPhysics.

Computes, per grid point column (nz=37):
  p     = a_k + b_k * ps                      (pressure)
  phi   = reverse-cumsum of R*T_mid*log_ratio (geopotential)
  omega = (b_k * S - cumsum(term)) / (ps+eps) (vertical velocity)

Strategy (8 NeuronCores, data-parallel over the 721x1440 spatial grid):
  - Host packs inputs k-major: 3 blocks of 37 levels -> 111 SBUF partitions,
    grid points along the free dim.
  - With a_k == 0 (always true for this problem) and ps >> EPS (EPS is below
    half-ulp of ps in fp32), the whole computation is LINEAR in the inputs
    along the level axis:
      phi   = G^T  @ T    (G folds R/2, log(b[k+1]/b[k]), adjacency, suffix sum)
      omega = M^T  @ dv   (M folds dsigma and b_k; ps cancels exactly in fp32)
      p     = Wp^T @ ps   (broadcast b_k * ps across levels)
    Three small block-diagonal fp32 matmuls per tile on the TensorEngine,
    evacuated PSUM->SBUF by ScalarE/VectorE, DMA'd out. Memory-bound.
  - If a_k != 0 or ps is small (EPS would matter), falls back to a numpy
    implementation of the reference (never triggered by the real inputs).
"""

import sys

import numpy as np

sys.path.insert(0, "/opt/trn_rl_repo")

R_DRY = 287.0
EPS = 1e-8

NZ = 37
NCORES = 8
NBLK = 3            # level-blocks stacked on the partition axis
PART = NBLK * NZ    # 111 partitions used (of 128)
FD = 512            # free-dim columns per tile == one PSUM bank of fp32
SUPER = 5           # compute tiles per DMA supertile (~1.1 MB per transfer)
NSUPER = 17         # supertiles per core
NTILES = SUPER * NSUPER  # 85 tiles per core
SFD = SUPER * FD    # 2560 columns per supertile
COLS = FD * NTILES  # 43520 grid points per block per core
PPC = NBLK * COLS   # 130560 grid points per core
MTOT = NCORES * PPC  # 1044480 padded total grid points

_state = {}


def _build_nc():
    import concourse.bacc as bacc
    import concourse.tile as tile
    from concourse import mybir

    f32 = mybir.dt.float32
    f32r = mybir.dt.float32r
    nc = bacc.Bacc()

    t_t = nc.dram_tensor("t_t", (PART, COLS), f32r, kind="ExternalInput")
    dv_t = nc.dram_tensor("dv_t", (PART, COLS), f32r, kind="ExternalInput")
    ps3 = nc.dram_tensor("ps3", (NBLK, COLS), f32, kind="ExternalInput")
    wp = nc.dram_tensor("wp", (NBLK, PART), f32, kind="ExternalInput")
    gmat = nc.dram_tensor("gmat", (PART, PART), f32r, kind="ExternalInput")
    mmat = nc.dram_tensor("mmat", (PART, PART), f32r, kind="ExternalInput")

    p_o = nc.dram_tensor("p_o", (PART, COLS), f32, kind="ExternalOutput")
    phi_o = nc.dram_tensor("phi_o", (PART, COLS), f32, kind="ExternalOutput")
    om_o = nc.dram_tensor("om_o", (PART, COLS), f32, kind="ExternalOutput")

    with tile.TileContext(nc) as tc:
        with (
            tc.tile_pool(name="consts", bufs=1) as cpool,
            tc.tile_pool(name="iin", bufs=4) as inpool,
            tc.tile_pool(name="iout", bufs=2) as outpool,
            tc.tile_pool(name="pssm", bufs=3) as pspool,
            tc.tile_pool(name="psum2", bufs=2, space="PSUM") as psum2,
            tc.tile_pool(name="psum3", bufs=3, space="PSUM") as psum3,
        ):
            wp_s = cpool.tile([NBLK, PART], f32)
            g_s = cpool.tile([PART, PART], f32r)
            m_s = cpool.tile([PART, PART], f32r)
            nc.sync.dma_start(wp_s[:], wp[:])
            nc.sync.dma_start(g_s[:], gmat[:])
            nc.sync.dma_start(m_s[:], mmat[:])

            # Software pipeline with prefetch distance 2: the gpsimd DMA
            # queue is FIFO, so a store that waits on compute would block
            # later loads (head-of-line). Emit loads 2 supertiles ahead.
            in_tiles = {}

            def load_super(s):
                ssl = slice(s * SFD, (s + 1) * SFD)
                ts_in = inpool.tile([PART, SFD], f32r, tag="ts_in")
                dv_in = inpool.tile([PART, SFD], f32r, tag="dv_in")
                nc.gpsimd.dma_start(ts_in[:], t_t[:, ssl])
                nc.gpsimd.dma_start(dv_in[:], dv_t[:, ssl])
                in_tiles[s] = (ts_in, dv_in)

            PREFETCH = 3
            for s in range(PREFETCH):
                load_super(s)

            for s in range(NSUPER):
                ssl = slice(s * SFD, (s + 1) * SFD)
                if s + PREFETCH < NSUPER:
                    load_super(s + PREFETCH)
                ts_in, dv_in = in_tiles.pop(s)

                p_out = outpool.tile([PART, SFD], f32, tag="p_out")
                phi_s = outpool.tile([PART, SFD], f32, tag="phi_s")
                om_s = outpool.tile([PART, SFD], f32, tag="om_s")

                for j in range(SUPER):
                    jsl = slice(j * FD, (j + 1) * FD)
                    i = s * SUPER + j
                    isl = slice(i * FD, (i + 1) * FD)

                    ps_in = pspool.tile([NBLK, FD], f32, tag="ps_in")
                    nc.sync.dma_start(ps_in[:], ps3[:, isl])

                    p_ps = psum2.tile([PART, FD], f32, tag="p_ps")
                    nc.tensor.matmul(p_ps[:], wp_s[:], ps_in[:])
                    nc.scalar.copy(p_out[:, jsl], p_ps[:])

                    phi_ps = psum3.tile([PART, FD], f32, tag="phi_ps")
                    nc.tensor.matmul(phi_ps[:], g_s[:], ts_in[:, jsl])
                    nc.scalar.copy(phi_s[:, jsl], phi_ps[:])

                    om_ps = psum3.tile([PART, FD], f32, tag="om_ps")
                    nc.tensor.matmul(om_ps[:], m_s[:], dv_in[:, jsl])
                    nc.vector.tensor_copy(om_s[:, jsl], om_ps[:])

                nc.scalar.dma_start(p_o[:, ssl], p_out[:])
                nc.sync.dma_start(phi_o[:, ssl], phi_s[:])
                nc.gpsimd.dma_start(om_o[:, ssl], om_s[:])

    nc.compile()
    return nc


def _get_nc():
    if "nc" not in _state:
        _state["nc"] = _build_nc()
    return _state["nc"]


def _pack(x_flat_padded):
    # [MTOT, NZ] -> [NCORES, PART, COLS]  (k-major, 3 point-blocks stacked)
    v = x_flat_padded.reshape(NCORES, NBLK, COLS, NZ).transpose(0, 1, 3, 2)
    return np.ascontiguousarray(v).reshape(NCORES, PART, COLS)


def _unpack(y, m0):
    # [NCORES, PART, COLS] -> [m0, NZ]
    v = y.reshape(NCORES, NBLK, NZ, COLS).transpose(0, 1, 3, 2)
    return np.ascontiguousarray(v).reshape(MTOT, NZ)[:m0]


def _weights(b_k, dsigma):
    b = b_k.astype(np.float64)
    ds = dsigma.astype(np.float64)
    logr = np.log(b[1:] / b[:-1])  # [36]

    # Wp[j, 37*j + k] = b[k]
    wp = np.zeros((NBLK, PART), np.float64)
    for j in range(NBLK):
        wp[j, 37 * j : 37 * (j + 1)] = b

    # phi[m] = sum_k G[k, m] * T[k]
    # G[k, m] = (R/2) * ( logr[k]   * [m <= k <= 35]
    #                   + logr[k-1] * [m+1 <= k <= 36] )
    g1 = np.zeros((NZ, NZ), np.float64)
    for m in range(NZ):
        for k in range(NZ):
            v = 0.0
            if m <= k <= 35:
                v += logr[k]
            if m + 1 <= k <= 36:
                v += logr[k - 1]
            g1[k, m] = 0.5 * R_DRY * v

    # omega[m] = sum_k M[k, m] * dv[k],  M[k, m] = dsigma[k]*(b[m] - [k <= m])
    m1 = ds[:, None] * (b[None, :] - (np.arange(NZ)[:, None] <= np.arange(NZ)[None, :]))

    gmat = np.zeros((PART, PART), np.float64)
    mmat = np.zeros((PART, PART), np.float64)
    for j in range(NBLK):
        s = slice(37 * j, 37 * (j + 1))
        gmat[s, s] = g1
        mmat[s, s] = m1
    return (
        wp.astype(np.float32),
        gmat.astype(np.float32),
        mmat.astype(np.float32),
    )


def _numpy_ref(ps, T, divergence, a_k, b_k, dsigma):
    p = a_k + b_k * ps[..., None]
    T_mid = 0.5 * (T[..., :-1] + T[..., 1:])
    log_ratio = np.log((p[..., 1:] + EPS) / (p[..., :-1] + EPS))
    dphi = R_DRY * T_mid * log_ratio
    rev_cumsum = np.cumsum(dphi[..., ::-1], axis=-1)[..., ::-1]
    phi = np.concatenate([rev_cumsum, np.zeros_like(p[..., :1])], axis=-1)
    term = ps[..., None] * divergence * dsigma
    col_integral = term.sum(axis=-1, keepdims=True)
    partial_sum = np.cumsum(term, axis=-1)
    omega = (b_k * col_integral - partial_sum) / (ps[..., None] + EPS)
    return np.stack([p, phi, omega], axis=0).astype(np.float32)


def run(inputs, trace=False):
    """Returns (full_output [3,B,NY,NX,NZ], exec_time_ns or None)."""
    ps = np.asarray(inputs["ps"], np.float32)
    T = np.asarray(inputs["T"], np.float32)
    dv = np.asarray(inputs["divergence"], np.float32)
    a_k = np.asarray(inputs["a_k"], np.float32)
    b_k = np.asarray(inputs["b_k"], np.float32)
    dsigma = np.asarray(inputs["dsigma"], np.float32)

    shp = T.shape  # [B, NY, NX, NZ]
    m0 = shp[0] * shp[1] * shp[2]

    # The fast path relies on a_k == 0 and EPS being below half-ulp of p/ps.
    if (
        np.any(a_k != 0.0)
        or ps.min() < 1.0
        or b_k.min() <= 0.0
        or shp[-1] != NZ
        or m0 > MTOT
    ):
        return _numpy_ref(ps, T, dv, a_k, b_k, dsigma), None

    from concourse.bass_utils import run_bass_kernel_spmd

    nc = _get_nc()
    wp, gmat, mmat = _weights(b_k, dsigma)

    pad = MTOT - m0
    Tf = np.concatenate([T.reshape(m0, NZ), np.zeros((pad, NZ), np.float32)])
    Df = np.concatenate([dv.reshape(m0, NZ), np.zeros((pad, NZ), np.float32)])
    Pf = np.concatenate([ps.reshape(m0), np.ones(pad, np.float32)])

    Tt = _pack(Tf)
    Dt = _pack(Df)
    P3 = np.ascontiguousarray(Pf.reshape(NCORES, NBLK, COLS))

    in_maps = [
        {
            "t_t": Tt[c],
            "dv_t": Dt[c],
            "ps3": P3[c],
            "wp": wp,
            "gmat": gmat,
            "mmat": mmat,
        }
        for c in range(NCORES)
    ]

    res = run_bass_kernel_spmd(
        nc, in_maps, core_ids=list(range(NCORES)), trace=trace
    )

    p_all = np.stack([res.results[c]["p_o"] for c in range(NCORES)])
    phi_all = np.stack([res.results[c]["phi_o"] for c in range(NCORES)])
    om_all = np.stack([res.results[c]["om_o"] for c in range(NCORES)])

    out = np.empty((3,) + shp, np.float32)
    out[0] = _unpack(p_all, m0).reshape(shp)
    out[1] = _unpack(phi_all, m0).reshape(shp)
    out[2] = _unpack(om_all, m0).reshape(shp)
    return out, res.exec_time_ns


def kernel(**inputs):
    return run(inputs, trace=False)[0]


# revision 12
# speedup vs baseline: 1.9450x; 1.9450x over previous
"""Trainium2 Bass kernel for nn_DiagnosticPhysics.

Computes, per grid point column (nz=37):
  p     = a_k + b_k * ps                      (pressure)
  phi   = reverse-cumsum of R*T_mid*log_ratio (geopotential)
  omega = (b_k * S - cumsum(term)) / (ps+eps) (vertical velocity)

Strategy (8 NeuronCores, data-parallel over the 721x1440 spatial grid):
  - Host packs inputs k-major: 3 blocks of 37 levels -> 111 SBUF partitions,
    grid points along the free dim.
  - With a_k == 0 (always true for this problem) and ps >> EPS (EPS is below
    half-ulp of ps in fp32), the whole computation is LINEAR in the inputs
    along the level axis:
      phi   = G^T  @ T    (G folds R/2, log(b[k+1]/b[k]), adjacency, suffix sum)
      omega = M^T  @ dv   (M folds dsigma and b_k; ps cancels exactly in fp32)
      p     = Wp^T @ ps   (broadcast b_k * ps across levels)
    Three small block-diagonal fp32 matmuls per tile on the TensorEngine,
    evacuated PSUM->SBUF by ScalarE/VectorE, DMA'd out. Memory-bound.
  - If a_k != 0 or ps is small (EPS would matter), falls back to a numpy
    implementation of the reference (never triggered by the real inputs).
"""

import sys

import numpy as np

sys.path.insert(0, "/opt/trn_rl_repo")

R_DRY = 287.0
EPS = 1e-8

NZ = 37
NCORES = 8
NBLK = 3            # level-blocks stacked on the partition axis
PART = NBLK * NZ    # 111 partitions used (of 128)
FD = 512            # free-dim columns per tile == one PSUM bank of fp32
SUPER = 5           # compute tiles per DMA supertile (~1.1 MB per transfer)
NSUPER = 17         # supertiles per core
NTILES = SUPER * NSUPER  # 85 tiles per core
SFD = SUPER * FD    # 2560 columns per supertile
COLS = FD * NTILES  # 43520 grid points per block per core
PPC = NBLK * COLS   # 130560 grid points per core
MTOT = NCORES * PPC  # 1044480 padded total grid points

_state = {}


def _build_nc():
    import concourse.bacc as bacc
    import concourse.tile as tile
    from concourse import mybir

    f32 = mybir.dt.float32
    f32r = mybir.dt.float32r
    nc = bacc.Bacc()

    t_t = nc.dram_tensor("t_t", (PART, COLS), f32r, kind="ExternalInput")
    dv_t = nc.dram_tensor("dv_t", (PART, COLS), f32r, kind="ExternalInput")
    ps3 = nc.dram_tensor("ps3", (NBLK, COLS), f32, kind="ExternalInput")
    wp = nc.dram_tensor("wp", (NBLK, PART), f32, kind="ExternalInput")
    gmat = nc.dram_tensor("gmat", (PART, PART), f32r, kind="ExternalInput")
    mmat = nc.dram_tensor("mmat", (PART, PART), f32r, kind="ExternalInput")

    p_o = nc.dram_tensor("p_o", (PART, COLS), f32, kind="ExternalOutput")
    phi_o = nc.dram_tensor("phi_o", (PART, COLS), f32, kind="ExternalOutput")
    om_o = nc.dram_tensor("om_o", (PART, COLS), f32, kind="ExternalOutput")

    with tile.TileContext(nc) as tc:
        with (
            tc.tile_pool(name="consts", bufs=1) as cpool,
            tc.tile_pool(name="iin", bufs=4) as inpool,
            tc.tile_pool(name="iout", bufs=2) as outpool,
            tc.tile_pool(name="pssm", bufs=3) as pspool,
            tc.tile_pool(name="psum2", bufs=2, space="PSUM") as psum2,
            tc.tile_pool(name="psum3", bufs=3, space="PSUM") as psum3,
        ):
            wp_s = cpool.tile([NBLK, PART], f32)
            g_s = cpool.tile([PART, PART], f32r)
            m_s = cpool.tile([PART, PART], f32r)
            nc.sync.dma_start(wp_s[:], wp[:])
            nc.sync.dma_start(g_s[:], gmat[:])
            nc.sync.dma_start(m_s[:], mmat[:])

            # Software pipeline with prefetch distance 2: the gpsimd DMA
            # queue is FIFO, so a store that waits on compute would block
            # later loads (head-of-line). Emit loads 2 supertiles ahead.
            in_tiles = {}

            def load_super(s):
                ssl = slice(s * SFD, (s + 1) * SFD)
                ts_in = inpool.tile([PART, SFD], f32r, tag="ts_in")
                dv_in = inpool.tile([PART, SFD], f32r, tag="dv_in")
                nc.gpsimd.dma_start(ts_in[:], t_t[:, ssl])
                nc.gpsimd.dma_start(dv_in[:], dv_t[:, ssl])
                in_tiles[s] = (ts_in, dv_in)

            PREFETCH = 3
            for s in range(PREFETCH):
                load_super(s)

            for s in range(NSUPER):
                ssl = slice(s * SFD, (s + 1) * SFD)
                if s + PREFETCH < NSUPER:
                    load_super(s + PREFETCH)
                ts_in, dv_in = in_tiles.pop(s)

                p_out = outpool.tile([PART, SFD], f32, tag="p_out")
                phi_s = outpool.tile([PART, SFD], f32, tag="phi_s")
                om_s = outpool.tile([PART, SFD], f32, tag="om_s")

                for j in range(SUPER):
                    jsl = slice(j * FD, (j + 1) * FD)
                    i = s * SUPER + j
                    isl = slice(i * FD, (i + 1) * FD)

                    ps_in = pspool.tile([NBLK, FD], f32, tag="ps_in")
                    nc.sync.dma_start(ps_in[:], ps3[:, isl])

                    p_ps = psum2.tile([PART, FD], f32, tag="p_ps")
                    nc.tensor.matmul(p_ps[:], wp_s[:], ps_in[:])
                    nc.scalar.copy(p_out[:, jsl], p_ps[:])

                    phi_ps = psum3.tile([PART, FD], f32, tag="phi_ps")
                    nc.tensor.matmul(phi_ps[:], g_s[:], ts_in[:, jsl])
                    nc.scalar.copy(phi_s[:, jsl], phi_ps[:])

                    om_ps = psum3.tile([PART, FD], f32, tag="om_ps")
                    nc.tensor.matmul(om_ps[:], m_s[:], dv_in[:, jsl])
                    nc.vector.tensor_copy(om_s[:, jsl], om_ps[:])

                nc.gpsimd.dma_start(p_o[:, ssl], p_out[:])
                nc.gpsimd.dma_start(phi_o[:, ssl], phi_s[:])
                nc.gpsimd.dma_start(om_o[:, ssl], om_s[:])

    nc.compile()
    return nc


def _get_nc():
    if "nc" not in _state:
        _state["nc"] = _build_nc()
    return _state["nc"]


def _pack(x_flat_padded):
    # [MTOT, NZ] -> [NCORES, PART, COLS]  (k-major, 3 point-blocks stacked)
    v = x_flat_padded.reshape(NCORES, NBLK, COLS, NZ).transpose(0, 1, 3, 2)
    return np.ascontiguousarray(v).reshape(NCORES, PART, COLS)


def _unpack(y, m0):
    # [NCORES, PART, COLS] -> [m0, NZ]
    v = y.reshape(NCORES, NBLK, NZ, COLS).transpose(0, 1, 3, 2)
    return np.ascontiguousarray(v).reshape(MTOT, NZ)[:m0]


def _weights(b_k, dsigma):
    b = b_k.astype(np.float64)
    ds = dsigma.astype(np.float64)
    logr = np.log(b[1:] / b[:-1])  # [36]

    # Wp[j, 37*j + k] = b[k]
    wp = np.zeros((NBLK, PART), np.float64)
    for j in range(NBLK):
        wp[j, 37 * j : 37 * (j + 1)] = b

    # phi[m] = sum_k G[k, m] * T[k]
    # G[k, m] = (R/2) * ( logr[k]   * [m <= k <= 35]
    #                   + logr[k-1] * [m+1 <= k <= 36] )
    g1 = np.zeros((NZ, NZ), np.float64)
    for m in range(NZ):
        for k in range(NZ):
            v = 0.0
            if m <= k <= 35:
                v += logr[k]
            if m + 1 <= k <= 36:
                v += logr[k - 1]
            g1[k, m] = 0.5 * R_DRY * v

    # omega[m] = sum_k M[k, m] * dv[k],  M[k, m] = dsigma[k]*(b[m] - [k <= m])
    m1 = ds[:, None] * (b[None, :] - (np.arange(NZ)[:, None] <= np.arange(NZ)[None, :]))

    gmat = np.zeros((PART, PART), np.float64)
    mmat = np.zeros((PART, PART), np.float64)
    for j in range(NBLK):
        s = slice(37 * j, 37 * (j + 1))
        gmat[s, s] = g1
        mmat[s, s] = m1
    return (
        wp.astype(np.float32),
        gmat.astype(np.float32),
        mmat.astype(np.float32),
    )


def _numpy_ref(ps, T, divergence, a_k, b_k, dsigma):
    p = a_k + b_k * ps[..., None]
    T_mid = 0.5 * (T[..., :-1] + T[..., 1:])
    log_ratio = np.log((p[..., 1:] + EPS) / (p[..., :-1] + EPS))
    dphi = R_DRY * T_mid * log_ratio
    rev_cumsum = np.cumsum(dphi[..., ::-1], axis=-1)[..., ::-1]
    phi = np.concatenate([rev_cumsum, np.zeros_like(p[..., :1])], axis=-1)
    term = ps[..., None] * divergence * dsigma
    col_integral = term.sum(axis=-1, keepdims=True)
    partial_sum = np.cumsum(term, axis=-1)
    omega = (b_k * col_integral - partial_sum) / (ps[..., None] + EPS)
    return np.stack([p, phi, omega], axis=0).astype(np.float32)


def run(inputs, trace=False):
    """Returns (full_output [3,B,NY,NX,NZ], exec_time_ns or None)."""
    ps = np.asarray(inputs["ps"], np.float32)
    T = np.asarray(inputs["T"], np.float32)
    dv = np.asarray(inputs["divergence"], np.float32)
    a_k = np.asarray(inputs["a_k"], np.float32)
    b_k = np.asarray(inputs["b_k"], np.float32)
    dsigma = np.asarray(inputs["dsigma"], np.float32)

    shp = T.shape  # [B, NY, NX, NZ]
    m0 = shp[0] * shp[1] * shp[2]

    # The fast path relies on a_k == 0 and EPS being below half-ulp of p/ps.
    if (
        np.any(a_k != 0.0)
        or ps.min() < 1.0
        or b_k.min() <= 0.0
        or shp[-1] != NZ
        or m0 > MTOT
    ):
        return _numpy_ref(ps, T, dv, a_k, b_k, dsigma), None

    from concourse.bass_utils import run_bass_kernel_spmd

    nc = _get_nc()
    wp, gmat, mmat = _weights(b_k, dsigma)

    pad = MTOT - m0
    Tf = np.concatenate([T.reshape(m0, NZ), np.zeros((pad, NZ), np.float32)])
    Df = np.concatenate([dv.reshape(m0, NZ), np.zeros((pad, NZ), np.float32)])
    Pf = np.concatenate([ps.reshape(m0), np.ones(pad, np.float32)])

    Tt = _pack(Tf)
    Dt = _pack(Df)
    P3 = np.ascontiguousarray(Pf.reshape(NCORES, NBLK, COLS))

    in_maps = [
        {
            "t_t": Tt[c],
            "dv_t": Dt[c],
            "ps3": P3[c],
            "wp": wp,
            "gmat": gmat,
            "mmat": mmat,
        }
        for c in range(NCORES)
    ]

    res = run_bass_kernel_spmd(
        nc, in_maps, core_ids=list(range(NCORES)), trace=trace
    )

    p_all = np.stack([res.results[c]["p_o"] for c in range(NCORES)])
    phi_all = np.stack([res.results[c]["phi_o"] for c in range(NCORES)])
    om_all = np.stack([res.results[c]["om_o"] for c in range(NCORES)])

    out = np.empty((3,) + shp, np.float32)
    out[0] = _unpack(p_all, m0).reshape(shp)
    out[1] = _unpack(phi_all, m0).reshape(shp)
    out[2] = _unpack(om_all, m0).reshape(shp)
    return out, res.exec_time_ns


def kernel(**inputs):
    return run(inputs, trace=False)[0]
